# revision 74
# baseline (speedup 1.0000x reference)
"""DenseCRFLoss on 8 Trainium2 NeuronCores.

Math: loss = -W/N * sum_k s_k^T K s_k per image, K[p,q] = exp(-0.5*||f_p-f_q||^2),
f = (x/50, y/50, rgb/15) on the 64x64 downsampled image, P=4096 pixels.

Device strategy (per core, SPMD-uniform program; data differs per core):
  - The PE produces y[p,q] = -G[p,q]/|ZLO| (the negated/rescaled log-kernel
    exponent, ~[0,1] over the clamp range) with ONE fp8e4 DoubleRow matmul
    per 512-col PSUM piece (0.5 PE cycles/column): features are multi-term
    fp8-split (4 terms rgb / 2 terms xy, 10/3 cross products kept), the
    negation/1/|ZLO| folded into the stat-side split, and the -|f|^2/2
    row/col terms enter as 2-term fp8 rows whose residual is folded into
    the segmentation weights on the host. 40 contraction rows, pair-packed
    into 20 partitions x 2 (DoubleRow layout).
  - exp runs on TWO engines in parallel: ACT banks use activation Exp with
    scale=64*ALPHA*ZLO*ln2; DVE banks use two custom-DVE ops: EXP2PY_ANT
    computes u = BETA*(1 - q(ZLO*y)) = ((D2 y + B2) y + A2) y on
    y = clip(in, 0, 1) -- zero constant term and free Zero/One clamps keep
    it inside the three scalar slots (the NRT DVE executor rejects
    Src1+imm2) -- written as uint16 (fixed point keeps 16-bit precision
    ahead of the ^64 amplification); EXP2QY_ANT computes q = C1 - u*C0
    (C1 absorbs the half-LSB truncation bias) and squares six times, so
    q^64 = 2^(64 ALPHA ZLO y) = K. Banks are assigned to the engines by a
    load-balancing greedy; an (optional, off by default) GPSIMD path can
    take over squaring chains for a few mid-stream banks.
  - The quadratic form uses "flipped" matmuls: E (bf16, SBUF) is the
    STATIONARY [128,128] and the per-block segmentation weights [128,2] are
    the moving side, so each 128-col chunk costs 2 PE cycles instead of 128+.
    Results accumulate in one shared PSUM bank (pending-zero first-touch
    gives each chunk a zeroed start). Host dots the [128, 18*4*2] result
    with column weights in fp64. Completed column ranges are copied out and
    DMA'd in three pieces so the final transfer is small.
  - Triangle harvesting: each image's 8x8 chunk grid upper triangle = 36
    quads; 2 cores per image, 18 quads each. Diagonal (straddle) quads
    compute only their upper-triangle pieces; diagonal 128x128 subtiles are
    computed UNMASKED but their flip uses a pre-halved aux column, so
    2*sum(partials) = full quadratic form. Straddle quads sit in adjacent
    slot pairs with piece orders chosen so banks pack exactly (no splits).
"""

import numpy as np
import ml_dtypes

WEIGHT = 2e-9
SIGMA_RGB = 15.0
SIGMA_XY = 100.0
SCALE = 0.5
LOG2E = float(np.log2(np.e))
LN2 = float(np.log(2.0))

NQ = 18
STRADDLE_SLOTS = (4, 5, 12, 13)   # adjacent pairs -> exact bank packing
# piece j-order inside a straddle pair: widths (512,384,256,128) interleave
# as 512 | 384+128 | 256+256 | 512 | 384+128  (5 banks exactly)
_STRADDLE_ORDER_A = (0, 1, 3, 2)
_STRADDLE_ORDER_B = (2, 0, 1, 3)
KP = 20                           # contraction partitions (40 rows / 2)
BANK = 512                        # PSUM bank width (fp32 cols)

_bf16 = ml_dtypes.bfloat16
_fp8 = ml_dtypes.float8_e4m3

# (stat_term, mov_term) index pairs per dim class (1-based split terms)
_PAIRS_XY = [(1, 1), (1, 2), (2, 1)]
_PAIRS_RGB = [(1, 1), (1, 2), (2, 1), (2, 2), (1, 3), (3, 1), (1, 4), (4, 1),
              (2, 3), (3, 2)]

_PROGRAM_CACHE = {}

# custom DVE exp2: q(z) = 1 + c z + c z^2 + d z^3 ~ 2^(ALPHA z) on
# z = ZLO*y with y = clip(-G/|ZLO|, 0, 1) (the host pre-negates/scales G so
# both clamps are the free Zero/One leaves).  EXP2PY emits
# u = BETA*(1 - q(ZLO*y)) = ((D2 y + B2) y + A2) y  in [0, 65535] as uint16
# (three scalar slots, no Src1 -- the NRT DVE executor rejects Src1+imm2);
# EXP2QY computes q = C1 - u*C0 (C1 absorbs the half-LSB truncation bias)
# and squares six times -> q^64 = 2^(64 ALPHA z) = K.
_ALPHA = 2.91
_ZLO = -0.1610824742268041
_QC = 2.0165895054552943
_QD = 1.1730176336542406
_U16 = 65535.0
_QLO = float((((_QD * _ZLO + _QC) * _ZLO + _QC) * _ZLO) + 1.0)  # q(ZLO)
_BETA = _U16 / (1.0 - _QLO)
_A2 = -_BETA * _QC * _ZLO
_B2 = -_BETA * _QC * _ZLO * _ZLO
_D2 = -_BETA * _QD * _ZLO * _ZLO * _ZLO
_EXP2_OPS = {}

# plan/program config. PSUM budget: act pools A+B (3+2 banks, alternating
# single-buffered groups = effective double buffering) + dve_w*dve_bufs +
# ubank <= 8.
_CFG = dict(act_widths=(3, 2), dve_w=1, dve_bufs=2, qbatch=4,
            lag=6, dve_skew=300.0, tail_act=1, warm_act=1,
            dve_adj=-20.0, act_adj=40.0, prewarm=0,
            pool_max=0, pool_chain=7200.0, pool_deadline=23000.0)


def _register_dve_exp2():
    if _EXP2_OPS:
        return _EXP2_OPS
    from concourse import dve_ops
    from concourse.dve_spec import (Spec, Src0, C0, C1, C2, C3, Zero, One,
                                    maxx, minn, sq, lower, spec_leaves,
                                    _spill_c3_to_src1, _has_src1)
    from concourse.dve_uop import DveOpSpec
    from concourse.dve_ops import DveOp

    def reg(name, spec):
        if name in dve_ops._SUB_OPCODE_FOR_NAME:
            _EXP2_OPS[name] = next(o for o in dve_ops.OPS if o.name == name)
            return
        row = dve_ops._CUSTOM_DVE_ROW_BASE + len(dve_ops.OPS)
        dve_ops._SUB_OPCODE_FOR_NAME[name] = row
        shas = {}
        for ver in ("v3", "v4"):
            s = DveOpSpec(name=name, opcode=row, uops=lower(spec, ver=ver),
                          rd1_en=_has_src1(spec))
            shas[ver] = s.sha(ver)
        op = DveOp(name, spec, subdim=False, uops_sha=shas)
        dve_ops.OPS.append(op)
        dve_ops.CUSTOM_DVE_SPECS[name] = op.spec
        _EXP2_OPS[name] = op

    # op1: y = clip(Src0, 0, 1); u = ((C0 y + C1) y + C2) y  (zero constant
    # term -> 3 scalar slots suffice; clamps use free Zero/One leaves).
    # 7 ALU stages; uint16 output (executor truncates toward zero).
    _y = minn(maxx(Src0, Zero), One)
    _body = ((C0 * _y + C1) * _y + C2) * _y

    def _ref_p(in0, in1, s0, s1, imm2):
        y = np.minimum(
            np.maximum(in0.astype(np.float32), np.float32(0.0)), np.float32(1.0))
        return ((np.float32(s0) * y + np.float32(s1)) * y + np.float32(imm2)) * y

    # op2: x = C1 - Src0 * C0 (C0 = 1/BETA, C1 = 1 - 0.5/BETA corrects the
    # truncation bias), then 6 squarings -> q^64.
    # 8 ALU stages; uint16 in, bf16 out, all SBUF -> eligible for 4x_2p.
    def _ref_s(in0, in1, s0, s1, imm2):
        x = np.float32(s1) - in0.astype(np.float32) * np.float32(s0)
        for _ in range(6):
            x = x * x
        return x

    reg("EXP2PY_ANT", Spec(body=_body, reference=_ref_p))
    reg("EXP2QY_ANT", Spec(body=sq(sq(sq(sq(sq(sq(C1 - Src0 * C0)))))),
                           reference=_ref_s))
    return _EXP2_OPS


def _quad_layout():
    """Per-image quad lists for the two cores sharing an image, plus the
    piece j-order per slot. Straddle (diagonal) quads land on
    STRADDLE_SLOTS (adjacent pairs)."""
    full = [(c, q) for c in range(8) for q in range(c)]  # 28
    stra = [(c, c) for c in range(8)]                    # 8

    orders = {}
    for k, s in enumerate(STRADDLE_SLOTS):
        orders[s] = _STRADDLE_ORDER_A if k % 2 == 0 else _STRADDLE_ORDER_B
    porder = [orders.get(s, (0, 1, 2, 3)) for s in range(NQ)]

    def arrange(fulls, stras):
        fi, si = iter(fulls), iter(stras)
        return [next(si) if s in STRADDLE_SLOTS else next(fi)
                for s in range(NQ)]

    even = arrange(full[0::2], stra[0:4])
    odd = arrange(full[1::2], stra[4:8])
    return even, odd, porder


def _plan(cfg=None):
    """Pack G pieces into a stream of 512-col PSUM banks, then group
    consecutive banks into per-engine exp instructions (load-balanced
    greedily).

    banks[b]: {pieces: [(slot, j, src_lo, width, bank_off)],
               flips:  [(slot, j, chunk, bank_off, aux_col, stop)]}
    groups[k]: {eng, b0, n}  - exp instr over banks [b0, b0+n)
    """
    cfg = cfg or _CFG
    _, _, porder = _quad_layout()

    pieces = []
    for i in range(NQ):
        if i in STRADDLE_SLOTS:
            for j in porder[i]:
                pieces.append((i, j, 128 * j, 512 - 128 * j))
        else:
            for j in range(4):
                pieces.append((i, j, 0, 512))

    # last piece (in emission order) writing each (slot, chunk) region
    last_writer = {}
    for pid, (i, j, lo, w) in enumerate(pieces):
        for cc in range(lo // 128, (lo + w) // 128):
            last_writer[(i, cc)] = pid

    si_of = {s: k for k, s in enumerate(STRADDLE_SLOTS)}
    banks = []
    cur = {"pieces": [], "flips": [], "fill": 0}
    for pid, (i, j, lo, w) in enumerate(pieces):
        while w > 0:
            if cur["fill"] == BANK:
                banks.append(cur)
                cur = {"pieces": [], "flips": [], "fill": 0}
            take = min(w, BANK - cur["fill"])
            off = cur["fill"]
            cur["pieces"].append((i, j, lo, take, off))
            for cc in range(lo // 128, (lo + take) // 128):
                if i in STRADDLE_SLOTS and cc == j:
                    aux_col = NQ * 8 + si_of[i] * 8 + j * 2   # halved (diag)
                else:
                    aux_col = i * 8 + j * 2
                cur["flips"].append((i, j, cc, off + (cc * 128 - lo), aux_col,
                                     last_writer[(i, cc)] == pid))
            cur["fill"] += take
            lo += take
            w -= take
    if cur["fill"]:
        banks.append(cur)
    nb = len(banks)

    # exp instr duration model (ns) for n banks: ACT does activation Exp,
    # DVE does EXP2PU (1x, fp32 from PSUM) + EXP2SU (4x_2p, u16->bf16 SBUF).
    def dur(eng, n):
        cols = n * BANK
        if eng == "act":
            return cols * 0.8333 + 225 + cfg.get("act_adj", 0.0)
        if eng == "dve_op1":   # pool group: DVE only pays op1
            return cols * 1.0417 + 185
        # op1 (fp32 PSUM read) + op2 (1x, batched over qbatch banks)
        return cols * (1.0417 + 1.0417) + 185 + cfg.get("dve_adj", 0.0)

    wA, wB = cfg["act_widths"]
    t = {"act": 0.0, "dve": cfg["dve_skew"]}
    groups = []
    b = 0
    act_count = 0
    pool_used = 0
    pool_t = 0.0

    def add_act(n):
        nonlocal b, act_count
        pool = act_count % 2          # 0 -> B (wB wide), 1 -> A (wA wide)
        cap = wA if pool == 1 else wB
        n = min(n, cap)
        groups.append({"eng": "act", "b0": b, "n": n, "pool": pool})
        t["act"] += dur("act", n)
        act_count += 1
        b += n

    warm = cfg.get("warm_act", 0)
    if warm:
        add_act(warm)
    while b < nb:
        if nb - b <= cfg["tail_act"]:
            # ACT drain at the tail (may exceed pool alternation width)
            n = min(wA, nb - b)
            groups.append({"eng": "act", "b0": b, "n": n,
                           "pool": act_count % 2 if n <= wB else 1})
            t["act"] += dur("act", n)
            act_count += 1
            b += n
            continue
        eng = min(t, key=lambda e: t[e])
        if eng == "act":
            add_act(wA if act_count % 2 == 1 else wB)
        else:
            n = min(cfg["dve_w"], nb - b)
            # offload this group's op2 (squarings) to the idle GPSIMD if its
            # slow serial chain still finishes well before the stream ends;
            # keep pool banks out of the first output-DMA piece (slots <= 4)
            slots_ok = all(p[0] > 4 for bb in range(b, b + n)
                           for p in banks[bb]["pieces"])
            use_pool = (pool_used < cfg["pool_max"] and slots_ok and
                        max(pool_t, t["dve"]) + cfg["pool_chain"] * n
                        <= cfg["pool_deadline"])
            groups.append({"eng": "dve", "b0": b, "n": n, "pool": 0,
                           "gpsimd": use_pool})
            if use_pool:
                pool_used += 1
                pool_t = max(pool_t, t["dve"] + dur("dve_op1", n)) \
                    + cfg["pool_chain"] * n
                t["dve"] += dur("dve_op1", n)
            else:
                t["dve"] += dur("dve", n)
            b += n

    # pool groups' flips are emitted AFTER the whole normal stream (their
    # e_t arrives from the slow GPSIMD chain); recompute per-(slot, chunk)
    # stop flags for that emission order.
    order = [bi for g in groups if not g.get("gpsimd")
             for bi in range(g["b0"], g["b0"] + g["n"])]
    order += [bi for g in groups if g.get("gpsimd")
              for bi in range(g["b0"], g["b0"] + g["n"])]
    last = {}
    for bi in order:
        for fi, f in enumerate(banks[bi]["flips"]):
            last[(f[0], f[2])] = (bi, fi)
    for bi in order:
        fl = banks[bi]["flips"]
        for fi, (i, j, cc, boff, acol, _stop) in enumerate(fl):
            fl[fi] = (i, j, cc, boff, acol, last[(i, cc)] == (bi, fi))
    return banks, groups


def _build_program(reps=1, cfg=None):
    import concourse.bacc as bacc
    import concourse.tile as tile
    from concourse import mybir

    cfg = cfg or _CFG
    nc = bacc.Bacc("TRN2", target_bir_lowering=False)
    dt = mybir.dt

    feat = nc.dram_tensor("feat", [KP, 2, NQ, 2, 512], dt.float8e4,
                          kind="ExternalInput")
    aux = nc.dram_tensor("aux", [128, NQ * 8 + 32], dt.bfloat16,
                         kind="ExternalInput")
    out = nc.dram_tensor("out", [128, NQ * 8], dt.float32, kind="ExternalOutput")

    banks, groups = _plan(cfg)
    ng = len(groups)
    wA, wB = cfg["act_widths"]
    dw = cfg["dve_w"]

    with tile.TileContext(nc) as tc:
        with (
            tc.tile_pool(name="consts", bufs=1) as consts,
            tc.tile_pool(name="gpaA", bufs=1, space="PSUM") as gpaA,
            tc.tile_pool(name="gpaB", bufs=1, space="PSUM") as gpaB,
            tc.tile_pool(name="gpd", bufs=cfg["dve_bufs"], space="PSUM") as gpd,
            tc.tile_pool(name="upool", bufs=1, space="PSUM") as upool,
            tc.tile_pool(name="epool", bufs=24) as epool,
            tc.tile_pool(name="qpool", bufs=3) as qpool,
        ):
            ops = _register_dve_exp2()
            exp2p, exp2s = ops["EXP2PY_ANT"], ops["EXP2QY_ANT"]
            feat_sb = consts.tile([KP, 2, NQ, 2, 512], dt.float8e4)
            aux_sb = consts.tile([128, NQ * 8 + 32], dt.bfloat16)
            warm = consts.tile([128, 1], dt.float32)

            # ACT table warm-up before any real dependency
            nc.vector.memset(warm, 0.0)
            nc.scalar.activation(out=warm[:, :], in_=warm[:, :],
                                 func=mybir.ActivationFunctionType.Exp,
                                 scale=64.0 * _ALPHA * _ZLO * LN2)

            # PE p-state prewarm: keep the PE continuously busy during the
            # input-DMA latency so the first real G fills run at full clock.
            npre = cfg.get("prewarm", 0)
            if npre:
                wsrc = consts.tile([KP, 2, 128], dt.float8e4)
                nc.vector.memset(wsrc, 0.0)
                wdst = upool.tile([128, 512], dt.float32, name="ubank")
                for _ in range(npre):
                    nc.tensor.matmul(
                        out=wdst[:, 384:512],
                        lhsT=wsrc[:, :, 0:128],
                        rhs=wsrc[:, :, 0:128],
                        start=True,
                        stop=True,
                        perf_mode=mybir.MatmulPerfMode.DoubleRow,
                    )

            # stage feat slot-planes in use order
            for (a, b) in ((0, 1), (1, 2), (2, 4)):
                nc.sync.dma_start(out=feat_sb[:, :, a:b], in_=feat[:, :, a:b])
            nc.sync.dma_start(out=aux_sb, in_=aux[:, :])
            for (a, b) in ((4, 8), (8, 13), (13, NQ)):
                nc.sync.dma_start(out=feat_sb[:, :, a:b], in_=feat[:, :, a:b])

            # output staging splits: once all banks holding pieces of slots
            # <= s are flipped, u cols 0:8*(s+1) are final and can be
            # copied + DMA'd out early (keeps the final copy tiny).
            def last_bank_for(s):
                return max(bi for bi, bank in enumerate(banks)
                           if any(p[0] <= s for p in bank["pieces"]))
            has_pool = any(g.get("gpsimd") for g in groups)
            if has_pool:
                # pool quads (slots >4) finalize last -> only slots <=4
                # ship from inside the stream
                splits = [[40, last_bank_for(4), False]]
            elif cfg.get("splits4", True):
                splits = [[72, last_bank_for(8), False],
                          [112, last_bank_for(13), False],
                          [136, last_bank_for(16), False]]
            else:
                splits = [[72, last_bank_for(8), False],
                          [112, last_bank_for(13), False]]

            for _rep in range(reps):
                ubank = upool.tile([128, 512], dt.float32, name="ubank")
                res_sb = consts.tile([128, NQ * 8], dt.float32)
                first_flip = [True]
                for sp in splits:
                    sp[2] = False

                def flips_for(group, e_t, ebase):
                    for bi in range(group["b0"], group["b0"] + group["n"]):
                        eoff = ebase + (bi - group["b0"]) * BANK
                        for (i, j, cc, boff, acol, stop) in banks[bi]["flips"]:
                            uidx = (i * 4 + cc) * 2
                            nc.tensor.matmul(
                                out=ubank[:, uidx:uidx + 2],
                                lhsT=e_t[:, eoff + boff:eoff + boff + 128],
                                rhs=aux_sb[:, acol:acol + 2],
                                start=first_flip[0],
                                stop=stop,
                                skip_group_check=True,
                            )
                            first_flip[0] = False
                    lo = 0
                    for sp in splits:
                        if not sp[2] and group["b0"] + group["n"] > sp[1]:
                            sp[2] = True
                            nc.vector.tensor_copy(res_sb[:, lo:sp[0]],
                                                  ubank[:, lo:sp[0]])
                            nc.sync.dma_start(out=out[:, lo:sp[0]],
                                              in_=res_sb[:, lo:sp[0]])
                        lo = sp[0]

                first_split = splits[-1][0]

                pending = []
                pending_pool = []
                cur_qb = [None]
                qb_max = cfg["qbatch"] * BANK
                et_max = max(qb_max, wA * BANK)

                def flush_qbatch(qb):
                    if qb is not None and qb["open"]:
                        nc.vector._custom_dve(
                            exp2s, out=qb["e"][:, 0:qb["fill"]],
                            in0=qb["q"][:, 0:qb["fill"]],
                            s0=1.0 / _BETA, s1=1.0 - 0.5 / _BETA)
                        qb["open"] = False
                        if qb is cur_qb[0]:
                            cur_qb[0] = None

                def pop_pending():
                    group, e_t, ebase, qb = pending.pop(0)
                    flush_qbatch(qb)   # flips below need this batch's op2
                    flips_for(group, e_t, ebase)

                last_dve_gk = max((i for i, g in enumerate(groups)
                                   if g["eng"] == "dve"), default=-1)
                for gk, group in enumerate(groups):
                    lag = min(cfg["lag"], max(1, ng - 1 - gk))
                    if gk == ng - 1:
                        flush_qbatch(cur_qb[0])
                    while len(pending) > lag:
                        pop_pending()
                    b0, n, eng = group["b0"], group["n"], group["eng"]
                    width = n * BANK
                    if eng == "act":
                        if group["pool"] == 1:
                            g_t = gpaA.tile([128, wA * BANK], dt.float32,
                                            name="g_actA")
                        else:
                            g_t = gpaB.tile([128, wB * BANK], dt.float32,
                                            name="g_actB")
                    else:
                        g_t = gpd.tile([128, dw * BANK], dt.float32, name="g_dve")
                    for bi in range(b0, b0 + n):
                        goff = (bi - b0) * BANK
                        for (i, j, lo, w, off) in banks[bi]["pieces"]:
                            nc.tensor.matmul(
                                out=g_t[:, goff + off:goff + off + w],
                                lhsT=feat_sb[:, 0, i, :, 128 * j:128 * (j + 1)],
                                rhs=feat_sb[:, 1, i, :, lo:lo + w],
                                start=True,
                                stop=True,
                                perf_mode=mybir.MatmulPerfMode.DoubleRow,
                            )
                    if eng == "act":
                        e_t = epool.tile([128, et_max], dt.bfloat16, name="e_t")
                        nc.scalar.activation(
                            out=e_t[:, 0:width], in_=g_t[:, 0:width],
                            func=mybir.ActivationFunctionType.Exp,
                            scale=64.0 * _ALPHA * _ZLO * LN2)
                        pending.append((group, e_t, 0, None))
                    elif group.get("gpsimd"):
                        # op1 on DVE, squaring chain on the idle GPSIMD;
                        # dedicated pools: these tiles live until the final
                        # flips, far longer than the main rings rotate
                        raise NotImplementedError(
                            "gpsimd path disabled (pool_max=0)")
                        nc.vector._custom_dve(
                            exp2p, out=q_t[:, 0:width], in0=g_t[:, 0:width],
                            s0=_D2, s1=_B2, imm2=_A2)
                        nc.gpsimd.tensor_scalar(
                            out=p_t[:, 0:width], in0=q_t[:, 0:width],
                            scalar1=-1.0 / _BETA, scalar2=1.0 - 0.5 / _BETA,
                            op0=mybir.AluOpType.mult,
                            op1=mybir.AluOpType.add)
                        for _sq in range(5):
                            nc.gpsimd.tensor_mul(out=p_t[:, 0:width],
                                                 in0=p_t[:, 0:width],
                                                 in1=p_t[:, 0:width])
                        nc.gpsimd.tensor_mul(out=e_t[:, 0:width],
                                             in0=p_t[:, 0:width],
                                             in1=p_t[:, 0:width])
                        pending_pool.append((group, e_t, 0))
                    else:
                        # a full batch flushes AFTER this group's op1 is
                        # emitted: the scheduler runs ready work in emission
                        # order, so the op1 goes first and the old batch's
                        # op2 pipeline-drain wait hides behind it
                        defer = None
                        if cur_qb[0] is not None and \
                                cur_qb[0]["fill"] >= qb_max:
                            defer = cur_qb[0]
                            cur_qb[0] = None
                        if cur_qb[0] is None:
                            cur_qb[0] = {
                                "q": qpool.tile([128, qb_max], dt.uint16,
                                                name="q_t"),
                                "e": epool.tile([128, et_max], dt.bfloat16,
                                                name="e_t"),
                                "fill": 0, "open": True,
                            }
                        qb = cur_qb[0]
                        ebase = qb["fill"]
                        nc.vector._custom_dve(
                            exp2p, out=qb["q"][:, ebase:ebase + width],
                            in0=g_t[:, 0:width],
                            s0=_D2, s1=_B2, imm2=_A2)
                        if defer is not None:
                            flush_qbatch(defer)
                        pending.append((group, qb["e"], ebase, qb))
                        qb["fill"] = ebase + width
                        if gk == last_dve_gk:
                            flush_qbatch(qb)
                flush_qbatch(cur_qb[0])
                while pending:
                    pop_pending()

                fin = splits[-1][0]
                nc.vector.tensor_copy(res_sb[:, fin:NQ * 8],
                                      ubank[:, fin:NQ * 8])
                nc.sync.dma_start(out=out[:, fin:NQ * 8],
                                  in_=res_sb[:, fin:NQ * 8])
                if has_pool:
                    for (pgrp, pe_t, pbase) in pending_pool:
                        flips_for(pgrp, pe_t, pbase)
                    nc.vector.tensor_copy(res_sb[:, first_split:112],
                                          ubank[:, first_split:112])
                    nc.sync.dma_start(out=out[:, first_split:112],
                                      in_=res_sb[:, first_split:112])

    nc.compile()
    return nc


def _get_program(reps=1):
    if reps not in _PROGRAM_CACHE:
        _PROGRAM_CACHE[reps] = _build_program(reps)
    return _PROGRAM_CACHE[reps]


def _split_fp8(x, n):
    terms = []
    r = np.asarray(x, np.float64).copy()
    for _ in range(n):
        t = r.astype(_fp8).astype(np.float64)
        terms.append(t)
        r = r - t
    return terms, r


def _prepare_inputs(images, segmentations):
    """Host-side shard/pack. Returns (in_maps, combine_info)."""
    N = images.shape[0]
    assert images.shape == (4, 3, 128, 128) and segmentations.shape == (4, 2, 128, 128)

    img = images[:, :, ::2, ::2].astype(np.float64)  # nearest, [4,3,64,64]

    s = segmentations.astype(np.float32)
    t = s[:, :, 0::2, :] * np.float32(0.5) + s[:, :, 1::2, :] * np.float32(0.5)
    seg = t[:, :, :, 0::2] * np.float32(0.5) + t[:, :, :, 1::2] * np.float32(0.5)
    seg = seg.reshape(N, 2, 4096).astype(np.float64)  # bilinear = 2x2 avg

    sxy = SIGMA_XY * SCALE
    yy, xx = np.meshgrid(np.arange(64.0), np.arange(64.0), indexing="ij")
    pos = np.stack([xx, yy], 0) / sxy
    feats = np.concatenate(
        [np.broadcast_to(pos[None], (N, 2, 64, 64)), img / SIGMA_RGB], axis=1
    ).reshape(N, 5, 4096)
    F = feats - feats.mean(axis=2, keepdims=True)
    F = F * np.sqrt(LOG2E / (64.0 * _ALPHA))     # exp2 poly units
    B = -0.5 * (F * F).sum(axis=1)               # [4,P]
    # y units: the G matmul produces y = -G/|ZLO| in [0, ~1]; fold the
    # negation into the STAT side and 1/|ZLO| into both sides.
    kappa = 1.0 / np.sqrt(-_ZLO)
    Fk = F * kappa
    Bny = B / _ZLO                               # = -B/|ZLO|, positive

    # per image: 40-row stat/mov stacks + corrected weights
    STAT = np.zeros((N, 40, 4096))
    MOV = np.zeros((N, 40, 4096))
    W = np.zeros((N, 2, 4096))
    for im in range(N):
        Fs, _ = _split_fp8(Fk[im], 4)            # Fs[t][5,P], 1-based below
        cs, delta = _split_fp8(Bny[im], 2)
        W[im] = seg[im] * np.exp2(64.0 * _ALPHA * _ZLO * delta)[None, :]
        r = 0
        for d in range(5):
            for (a, b) in (_PAIRS_XY if d < 2 else _PAIRS_RGB):
                STAT[im, r] = -Fs[a - 1][d]
                MOV[im, r] = Fs[b - 1][d]
                r += 1
        for c in cs:                              # stat c rows, moving ones
            STAT[im, r] = c
            MOV[im, r] = 1.0
            r += 1
        for c in cs:                              # stat ones, moving c cols
            STAT[im, r] = 1.0
            MOV[im, r] = c
            r += 1
        assert r == 40

    W_bf = W.astype(_bf16)
    Wh_bf = (W * 0.5).astype(_bf16)              # halved (diagonal flips)

    even, odd, _ = _quad_layout()
    si_of = {sl: k for k, sl in enumerate(STRADDLE_SLOTS)}
    in_maps = []
    wcols = []
    for core in range(8):
        im = core // 2
        quads = even if core % 2 == 0 else odd
        feat_arr = np.zeros((KP, 2, NQ, 2, 512), _fp8)
        aux_arr = np.zeros((128, NQ * 8 + 32), _bf16)
        wcol = np.zeros((NQ, 4, 2, 128))
        for slot, (c, q) in enumerate(quads):
            feat_arr[:, 0, slot] = \
                STAT[im][:, 512 * q:512 * (q + 1)].reshape(KP, 2, 512).astype(_fp8)
            feat_arr[:, 1, slot] = \
                MOV[im][:, 512 * c:512 * (c + 1)].reshape(KP, 2, 512).astype(_fp8)
            for j in range(4):
                rlo = 512 * q + 128 * j
                aux_arr[:, slot * 8 + j * 2:slot * 8 + j * 2 + 2] = \
                    W_bf[im][:, rlo:rlo + 128].T
                if slot in STRADDLE_SLOTS:
                    col = NQ * 8 + si_of[slot] * 8 + j * 2
                    aux_arr[:, col:col + 2] = Wh_bf[im][:, rlo:rlo + 128].T
            for cc in range(4):
                clo = 512 * c + 128 * cc
                wcol[slot, cc] = W_bf[im][:, clo:clo + 128].astype(np.float64)
        in_maps.append({"feat": np.ascontiguousarray(feat_arr),
                        "aux": np.ascontiguousarray(aux_arr)})
        wcols.append(wcol)
    return in_maps, wcols


def _combine(outs, wcols, n_images=4):
    total = 0.0
    for core, o in enumerate(outs):
        u = np.asarray(o["out"], np.float64).reshape(128, NQ, 4, 2)
        # sum_m u[m, slot, chunk, k] * wcol[slot, chunk, k, m]
        total += np.einsum("mick,ickm->", u, wcols[core])
    loss = -WEIGHT * 2.0 * total / n_images
    return np.array([loss], dtype=np.float32)


def kernel(images, segmentations):
    import time as _time
    from concourse.bass_utils import run_bass_kernel_spmd

    in_maps, wcols = _prepare_inputs(np.asarray(images), np.asarray(segmentations))
    nc = _get_program(reps=1)
    last_err = None
    for attempt in range(8):  # the NRT backend occasionally fails transiently
        try:
            res = run_bass_kernel_spmd(nc, in_maps, core_ids=list(range(8)))
            return _combine(res.results, wcols)
        except Exception as e:  # noqa: BLE001
            last_err = e
            _time.sleep(0.5 * (attempt + 1))
    raise last_err
